# revision 14
# baseline (speedup 1.0000x reference)
"""Inverse Daubechies (db4) wavelet layer on 8 Trainium2 NeuronCores.

Math: input [16, 16000, 128] splits into approx (ch 0:64) / detail (ch 64:128).
Each half is zero-upsampled 2x along L and cross-correlated with an 8-tap
filter (TF SAME padding, pad_left=3), outputs summed -> [16, 32000, 64].

Polyphase view: out[2t]   = sum_j rec[2j+1] * z[t+j-1]
               out[2t+1] = sum_j rec[2j]   * z[t+j-1]        (j = 0..3)
summed over both halves (rec_lo on approx + rec_hi on detail).

Kernel strategy (per core): shard L into 8 slices of 2000 input rows.
The whole upsample+conv+sum is expressed as PE matmuls with banded
stationary matrices: partition dim = input L-rows (K=128 window), free dim
= (batch, channel) (N=512), M = 125 output positions per phase. PSUM
accumulation fuses the approx+detail sum. Even/odd phases are copied into
an SBUF tile so each partition holds a consecutive output row pair ->
fully contiguous per-partition DMA to DRAM.

I/O is bf16 end to end (inputs quantized host-side, outputs upcast
host-side): the end-to-end latency is dominated by host<->device
transfer, and bf16 halves every leg while staying well inside the
accuracy budget. The PJRT execution path is memoized per program so
repeated calls reuse the loaded executable, and the donated output
buffers are created device-side instead of being uploaded.
"""

import numpy as np
import ml_dtypes

import concourse.bass as bass
import concourse.tile as tile
from concourse import mybir
from concourse.bass_utils import run_bass_kernel_spmd
from concourse.vector_clock import ScopedClock, VectorClock

F32 = mybir.dt.float32
BF16 = mybir.dt.bfloat16
NP_BF16 = ml_dtypes.bfloat16

N_CORES = 8
NB = 16        # batches
CIN = 128      # input channels (64 approx + 64 detail)
C = 64         # output channels
L = 16000      # input length
ROWS_PER_CORE = L // N_CORES          # 2000
WINDOWS = ROWS_PER_CORE // 125        # 16 windows of 125 rows
XROWS = ROWS_PER_CORE + 3             # 2003 padded rows per core


class _TileContextFixed(tile.TileContext):
    """This walrus build only encodes one sync wait per instruction; Tile's
    final drain carries one wait per logical proc. Split them into
    single-wait nops ahead of a waitless drain."""

    def _drain_and_barrier(self, tick_clock, wait_clock):
        nc = self.nc
        gc = tick_clock.global_clock
        n = len(gc)
        for p in range(n):
            t = gc[p]
            if t <= 0:
                continue
            vec = [0] * n
            vec[p] = t
            nop = nc.sync.nop(nofuse=True, hint=f"drain_wait_p{p}")
            wait_clock.add_sem_waits(nop.ins, ScopedClock({None: VectorClock(vec)}))
        nc.sync.drain()
        nc.all_engine_barrier()
        assert self.sems is not None
        popped = nc._tile_sem_poison_stack.pop()
        assert popped is self._sem_poison
        nc.clear_and_free_semaphores(list(self.sems.allocated().values()))
        nc.all_engine_barrier()


def _build_program():
    nc = bass.Bass(
        trn_type="TRN2", target_bir_lowering=False, debug=False, num_devices=N_CORES
    )
    x = nc.dram_tensor("x", (NB, XROWS, CIN), BF16, kind="ExternalInput")
    s = nc.dram_tensor("s", (128, 500), BF16, kind="ExternalInput")
    y = nc.dram_tensor("y", (NB, 2 * ROWS_PER_CORE, C), BF16, kind="ExternalOutput")

    with _TileContextFixed(nc) as tc:
        with (
            tc.tile_pool(name="const", bufs=1) as cpool,
            tc.tile_pool(name="xin", bufs=3) as xpool,
            tc.tile_pool(name="outb", bufs=3) as opool,
            tc.tile_pool(name="ps", bufs=8, space="PSUM") as pspool,
        ):
            s_sb = cpool.tile([128, 500], BF16)
            nc.sync.dma_start(s_sb[:], s[:])
            s_ea = s_sb[:, 0:125]
            s_ed = s_sb[:, 125:250]
            s_oa = s_sb[:, 250:375]
            s_od = s_sb[:, 375:500]

            for i in range(WINDOWS):
                xt = xpool.tile([128, NB, CIN], BF16)
                nc.sync.dma_start(
                    xt[:], x[:, 125 * i : 125 * i + 128, :].rearrange("b r c -> r b c")
                )
                a_h = [xt[:, 8 * h : 8 * h + 8, 0:C] for h in range(2)]
                d_h = [xt[:, 8 * h : 8 * h + 8, C:CIN] for h in range(2)]
                ps_e = [pspool.tile([128, 8, C], F32, tag="ps", name=f"ps_e{i}_{h}") for h in range(2)]
                ps_o = [pspool.tile([128, 8, C], F32, tag="ps", name=f"ps_o{i}_{h}") for h in range(2)]
                # weight-paired order: 4 stationary loads per window
                for h in range(2):
                    nc.tensor.matmul(ps_e[h][0:125], s_ea, a_h[h], start=True, stop=False)
                for h in range(2):
                    nc.tensor.matmul(ps_e[h][0:125], s_ed, d_h[h], start=False, stop=True)
                for h in range(2):
                    nc.tensor.matmul(ps_o[h][0:125], s_oa, a_h[h], start=True, stop=False)
                for h in range(2):
                    nc.tensor.matmul(ps_o[h][0:125], s_od, d_h[h], start=False, stop=True)

                ot = opool.tile([128, NB, 2, C], BF16)
                nc.scalar.copy(ot[0:125, 0:8, 0, :], ps_e[0][0:125])
                nc.scalar.copy(ot[0:125, 0:8, 1, :], ps_o[0][0:125])
                nc.vector.tensor_copy(ot[0:125, 8:16, 0, :], ps_e[1][0:125])
                nc.vector.tensor_copy(ot[0:125, 8:16, 1, :], ps_o[1][0:125])

                nc.scalar.dma_start(
                    y[:, 250 * i : 250 * i + 250, :].rearrange(
                        "b (q two) c -> q b (two c)", two=2
                    ),
                    ot[0:125].rearrange("p b s c -> p b (s c)"),
                )
    _install_wait_splitter(nc)
    return nc


def _install_wait_splitter(nc):
    """This walrus build encodes at most one sync wait per instruction. Split
    every multi-wait instruction in the serialized BIR into single-wait NoOps
    placed immediately before it on the same engine (in-order semantics are
    identical)."""
    import orjson

    orig = nc.to_json_bytes

    def patched():
        d = orjson.loads(orig())
        n_split = 0
        for fn in d["functions"]:
            for bb in fn["blocks"]:
                out = []
                for inst in bb["instructions"]:
                    si = inst.get("sync_info")
                    waits = si.get("on_wait", []) if si else []
                    if len(waits) > 1:
                        for j, w in enumerate(waits[:-1]):
                            out.append(
                                {
                                    "debug": inst.get("debug", 0),
                                    "engine": inst["engine"],
                                    "ins": [],
                                    "name": f"{inst['name']}_sw{j}",
                                    "opcode": "NoOp",
                                    "outs": [],
                                    "sync_info": {
                                        "on_update": [],
                                        "on_wait": [w],
                                    },
                                    "text_hint": "split_wait",
                                }
                            )
                            n_split += 1
                        si["on_wait"] = [waits[-1]]
                    out.append(inst)
                bb["instructions"] = out
        return orjson.dumps(d)

    nc.to_json_bytes = patched


# ---------------------------------------------------------------------------
# Fast PJRT execution path: functionally identical to
# concourse.bass2jax.run_bass_via_pjrt, but (a) the traced/lowered/compiled
# executable is memoized per Bass program instead of being rebuilt (and
# recompiled) on every call, and (b) the donated output buffers are created
# on-device by a tiny jitted fill instead of uploading host zero arrays
# through the tunnel. Data semantics per call are unchanged: inputs are
# uploaded, the NEFF runs on all cores, outputs are fetched.
# ---------------------------------------------------------------------------

_PJRT_CACHE = {}


def _fast_run_bass_via_pjrt(nc, in_maps, n_cores):
    import jax
    import jax.numpy as jnp
    from jax.sharding import Mesh, NamedSharding, PartitionSpec

    from jax.experimental.shard_map import shard_map
    from concourse import bass2jax

    if nc.dbg_addr is not None or n_cores == 1:
        return _ORIG_RUN_VIA_PJRT(nc, in_maps, n_cores)

    entry = _PJRT_CACHE.get(id(nc))
    if entry is None:
        bass2jax.install_neuronx_cc_hook()
        partition_name = (
            nc.partition_id_tensor.name if nc.partition_id_tensor else None
        )
        in_names, out_names, out_avals = [], [], []
        for alloc in nc.m.functions[0].allocations:
            if not isinstance(alloc, mybir.MemoryLocationSet):
                continue
            assert alloc.memorylocations
            name = alloc.memorylocations[0].name
            if alloc.kind == "ExternalInput":
                if name != partition_name:
                    in_names.append(name)
            elif alloc.kind == "ExternalOutput":
                assert alloc.tensor_shape is not None and alloc.dtype is not None
                out_names.append(name)
                out_avals.append(
                    jax.core.ShapedArray(
                        tuple(alloc.tensor_shape), mybir.dt.np(alloc.dtype)
                    )
                )
        n_params = len(in_names)
        n_outs = len(out_avals)
        all_in_names = in_names + out_names
        if partition_name is not None:
            all_in_names.append(partition_name)
        donate = tuple(range(n_params, n_params + n_outs))

        def _body(*args):
            operands = list(args)
            if partition_name is not None:
                operands.append(bass2jax.partition_id_tensor())
            outs = bass2jax._bass_exec_p.bind(
                *operands,
                out_avals=tuple(out_avals),
                in_names=tuple(all_in_names),
                out_names=tuple(out_names),
                lowering_input_output_aliases=(),
                sim_require_finite=True,
                sim_require_nnan=True,
                nc=nc,
            )
            return tuple(outs)

        devices = jax.devices()[:n_cores]
        assert len(devices) == n_cores
        mesh = Mesh(np.asarray(devices), ("core",))
        in_specs = (PartitionSpec("core"),) * (n_params + n_outs)
        out_specs = (PartitionSpec("core"),) * n_outs
        sharded = jax.jit(
            shard_map(
                _body,
                mesh=mesh,
                in_specs=in_specs,
                out_specs=out_specs,
                check_rep=False,
            ),
            donate_argnums=donate,
            keep_unused=True,
        )
        gshapes = [
            (n_cores * av.shape[0], *av.shape[1:]) for av in out_avals
        ]
        gdtypes = [av.dtype for av in out_avals]
        shd = NamedSharding(mesh, PartitionSpec("core"))
        make_zeros = jax.jit(
            lambda: tuple(
                jnp.zeros(gs, gd) for gs, gd in zip(gshapes, gdtypes)
            ),
            out_shardings=(shd,) * n_outs,
        )
        entry = (in_names, out_names, out_avals, n_params, sharded, make_zeros, shd, {})
        _PJRT_CACHE[id(nc)] = entry

    in_names, out_names, out_avals, n_params, sharded, make_zeros, shd, small_cache = entry
    import os as _os
    import time as _time

    _timing = _os.environ.get("KM_TIMING", "0") == "1"
    _t0 = _time.time()
    concat_in = [
        _concat_or_base([np.asarray(m[name]) for m in in_maps]) for name in in_names
    ]
    # Explicit sharded upload: device_put(global, NamedSharding) streams
    # measurably faster than the jit argument-processing path, and small
    # constant inputs (e.g. filter matrices) stay device-resident keyed by
    # content so repeated calls skip their upload entirely.
    dev_in = []
    for name, arr in zip(in_names, concat_in):
        if arr.nbytes <= (4 << 20):
            key = (name, arr.shape, arr.dtype.str, arr.tobytes())
            buf = small_cache.get(key)
            if buf is None:
                small_cache.clear()
                buf = jax.device_put(arr, shd)
                small_cache[key] = buf
            dev_in.append(buf)
        else:
            dev_in.append(jax.device_put(arr, shd))
    _t1 = _time.time()
    zeros = make_zeros()
    _t2 = _time.time()
    out_arrs = sharded(*dev_in, *zeros)
    for o in out_arrs:
        o.block_until_ready()
    _t3 = _time.time()
    fetched = [
        _fetch_sharded(o, av, n_cores, nm)
        for o, av, nm in zip(out_arrs, out_avals, out_names)
    ]
    _t4 = _time.time()
    if _timing:
        print(
            f"[km] concat+upload={_t1 - _t0:.3f}s zeros={_t2 - _t1:.3f}s "
            f"exec={_t3 - _t2:.3f}s fetch={_t4 - _t3:.3f}s"
        )
    return [
        {
            name: fetched[i][c]
            for i, name in enumerate(out_names)
        }
        for c in range(n_cores)
    ]


def _concat_or_base(arrs):
    """Concatenate per-core arrays along axis 0 — zero-copy when they are
    consecutive contiguous views of one base array of exactly that shape."""
    base = arrs[0].base
    if (
        base is not None
        and all(a.base is base for a in arrs)
        and all(a.flags["C_CONTIGUOUS"] for a in arrs)
        and base.flags["C_CONTIGUOUS"]
        and base.dtype == arrs[0].dtype
        and base.size == sum(a.size for a in arrs)
    ):
        ptr = base.__array_interface__["data"][0]
        ok = True
        for a in arrs:
            if a.__array_interface__["data"][0] != ptr:
                ok = False
                break
            ptr += a.nbytes
        if ok:
            gshape = (sum(a.shape[0] for a in arrs),) + arrs[0].shape[1:]
            return base.reshape(gshape)
    return np.concatenate(arrs, axis=0)


_FETCH_POOL = None

# Optional per-call fetch sink set by kernel(): {out_name: fn(core)->dst_view}.
# When present, each fetched shard is cast-assigned straight into the caller's
# preallocated final buffer inside the fetch thread (overlapping the tunnel
# round-trips with the bf16->f32 cast) instead of staging a host copy.
_OUT_SINK = None


def _fetch_sharded(arr, aval, n_cores, name):
    """Fetch a fully core-sharded device array with one thread per shard
    (overlaps the per-shard tunnel round-trips). Returns a per-core indexable
    of host arrays."""
    global _FETCH_POOL
    from concurrent.futures import ThreadPoolExecutor

    shards = sorted(
        arr.addressable_shards, key=lambda s: s.index[0].start or 0
    )
    if len(shards) != n_cores:
        return np.asarray(arr).reshape(n_cores, *aval.shape)
    if _FETCH_POOL is None:
        _FETCH_POOL = ThreadPoolExecutor(n_cores)
    sink = _OUT_SINK.get(name) if _OUT_SINK else None
    if sink is None:
        out = np.empty((n_cores, *aval.shape), aval.dtype)

        def _pull(c):
            out[c] = np.asarray(shards[c].data)

        list(_FETCH_POOL.map(_pull, range(n_cores)))
        return out

    res = [None] * n_cores

    def _pull_sink(c):
        dst = sink(c)
        dst[...] = np.asarray(shards[c].data)
        res[c] = dst

    list(_FETCH_POOL.map(_pull_sink, range(n_cores)))
    return res


_ORIG_RUN_VIA_PJRT = None


def _install_fast_pjrt():
    global _ORIG_RUN_VIA_PJRT
    from concourse import bass2jax

    if _ORIG_RUN_VIA_PJRT is None:
        _ORIG_RUN_VIA_PJRT = bass2jax.run_bass_via_pjrt
        bass2jax.run_bass_via_pjrt = _fast_run_bass_via_pjrt


_NC = None


def _get_nc():
    global _NC
    if _NC is None:
        _NC = _build_program()
    return _NC


def _band_matrices(rec_lo: np.ndarray, rec_hi: np.ndarray) -> np.ndarray:
    """[128, 500] = [S_even_approx | S_even_detail | S_odd_approx | S_odd_detail].

    S[k, m]: coefficient linking input row r0+k to output pair m of a window
    (k = m + j, j = 0..3). Even phase uses taps f[2j+1], odd phase f[2j]."""
    s = np.zeros((128, 500), np.float32)
    lo = np.asarray(rec_lo, np.float32)
    hi = np.asarray(rec_hi, np.float32)
    for m in range(125):
        for j in range(4):
            k = m + j
            s[k, m] = lo[2 * j + 1]
            s[k, 125 + m] = hi[2 * j + 1]
            s[k, 250 + m] = lo[2 * j]
            s[k, 375 + m] = hi[2 * j]
    return s.astype(NP_BF16)


def _shard_inputs(x_f32: np.ndarray, s: np.ndarray) -> list:
    """Per-core input maps: 2003 rows each (1-row left halo, 2-row right),
    zero-padded at the global edges, cast f32->bf16 during the slab build.
    The per-core slabs are consecutive views of one preallocated global array
    so the pjrt path can skip its concat."""
    xg = np.empty((N_CORES * NB, XROWS, CIN), NP_BF16)
    for core in range(N_CORES):
        r0 = ROWS_PER_CORE * core
        xc = xg[core * NB : (core + 1) * NB]
        if core == 0:
            xc[:, 0, :] = 0
            xc[:, 1:, :] = x_f32[:, 0 : XROWS - 1, :]
        elif core == N_CORES - 1:
            xc[:, XROWS - 2 :, :] = 0
            xc[:, : XROWS - 2, :] = x_f32[:, r0 - 1 : L, :]
        else:
            xc[:] = x_f32[:, r0 - 1 : r0 - 1 + XROWS, :]
    return [
        {"x": xg[core * NB : (core + 1) * NB], "s": s} for core in range(N_CORES)
    ]


def kernel(inputs: np.ndarray, rec_lo: np.ndarray, rec_hi: np.ndarray) -> np.ndarray:
    global _OUT_SINK
    import os as _os
    import time as _time

    _timing = _os.environ.get("KM_TIMING", "0") == "1"
    _t0 = _time.time()
    inputs = np.asarray(inputs, np.float32)
    assert inputs.shape == (NB, L, CIN), inputs.shape
    _install_fast_pjrt()
    nc = _get_nc()
    s = _band_matrices(rec_lo, rec_hi)
    in_maps = _shard_inputs(inputs, s)
    out = np.empty((NB, 2 * L, C), np.float32)

    def _y_dst(core):
        sl = slice(2 * ROWS_PER_CORE * core, 2 * ROWS_PER_CORE * (core + 1))
        return out[:, sl, :]

    _t1 = _time.time()
    _OUT_SINK = {"y": _y_dst}
    try:
        res = run_bass_kernel_spmd(nc, in_maps, list(range(N_CORES)))
    finally:
        _OUT_SINK = None
    _t2 = _time.time()
    # If the fetch sink was honored, the per-core results are views into
    # `out` already; otherwise (fallback paths) cast-assign them now.
    for core in range(N_CORES):
        got = res.results[core]["y"]
        if not np.shares_memory(got, out):
            _y_dst(core)[...] = got
    _t3 = _time.time()
    if _timing:
        print(
            f"[km kernel] prep={_t1 - _t0:.3f}s spmd={_t2 - _t1:.3f}s "
            f"reassemble={_t3 - _t2:.3f}s"
        )
    return out


# revision 15
# speedup vs baseline: 1.2599x; 1.2599x over previous
"""Inverse Daubechies (db4) wavelet layer on 8 Trainium2 NeuronCores.

Math: input [16, 16000, 128] splits into approx (ch 0:64) / detail (ch 64:128).
Each half is zero-upsampled 2x along L and cross-correlated with an 8-tap
filter (TF SAME padding, pad_left=3), outputs summed -> [16, 32000, 64].

Polyphase view: out[2t]   = sum_j rec[2j+1] * z[t+j-1]
               out[2t+1] = sum_j rec[2j]   * z[t+j-1]        (j = 0..3)
summed over both halves (rec_lo on approx + rec_hi on detail).

Kernel strategy (per core, per chunk): shard L across the 8 cores; each
chunk covers ROWS_PER_CORE input rows per core. The upsample+conv+sum is
expressed as PE matmuls with banded stationary matrices: partition dim =
input L-rows (K=128 window), free dim = (batch, channel) (N=512), M = 125
output positions per phase. PSUM accumulation fuses the approx+detail sum.
Even/odd phases are copied into an SBUF tile so each partition holds a
consecutive output row pair -> fully contiguous per-partition DMA to DRAM.

Host/transfer strategy: the end-to-end latency is dominated by the
host<->device tunnel, so I/O is bf16 end to end (inputs quantized during
the shard build, outputs cast to f32 inside the fetch threads), the PJRT
execution path is memoized per program, donated output buffers are created
device-side, and the L range is split into CHUNKS sequential spmd calls
pipelined on two threads so chunk k+1's upload overlaps chunk k's fetch.
"""

import threading
import numpy as np
import ml_dtypes

import concourse.bass as bass
import concourse.tile as tile
from concourse import mybir
from concourse.bass_utils import run_bass_kernel_spmd
from concourse.vector_clock import ScopedClock, VectorClock

F32 = mybir.dt.float32
BF16 = mybir.dt.bfloat16
NP_BF16 = ml_dtypes.bfloat16

N_CORES = 8
NB = 16        # batches
CIN = 128      # input channels (64 approx + 64 detail)
C = 64         # output channels
L = 16000      # input length
CHUNKS = 2     # sequential pipelined spmd calls over the L axis


class _TileContextFixed(tile.TileContext):
    """This walrus build only encodes one sync wait per instruction; Tile's
    final drain carries one wait per logical proc. Split them into
    single-wait nops ahead of a waitless drain."""

    def _drain_and_barrier(self, tick_clock, wait_clock):
        nc = self.nc
        gc = tick_clock.global_clock
        n = len(gc)
        for p in range(n):
            t = gc[p]
            if t <= 0:
                continue
            vec = [0] * n
            vec[p] = t
            nop = nc.sync.nop(nofuse=True, hint=f"drain_wait_p{p}")
            wait_clock.add_sem_waits(nop.ins, ScopedClock({None: VectorClock(vec)}))
        nc.sync.drain()
        nc.all_engine_barrier()
        assert self.sems is not None
        popped = nc._tile_sem_poison_stack.pop()
        assert popped is self._sem_poison
        nc.clear_and_free_semaphores(list(self.sems.allocated().values()))
        nc.all_engine_barrier()


def _build_program(rows_per_core):
    windows = rows_per_core // 125
    xrows = rows_per_core + 3
    nc = bass.Bass(
        trn_type="TRN2", target_bir_lowering=False, debug=False, num_devices=N_CORES
    )
    x = nc.dram_tensor("x", (NB, xrows, CIN), BF16, kind="ExternalInput")
    s = nc.dram_tensor("s", (128, 500), BF16, kind="ExternalInput")
    y = nc.dram_tensor("y", (NB, 2 * rows_per_core, C), BF16, kind="ExternalOutput")

    with _TileContextFixed(nc) as tc:
        with (
            tc.tile_pool(name="const", bufs=1) as cpool,
            tc.tile_pool(name="xin", bufs=3) as xpool,
            tc.tile_pool(name="outb", bufs=3) as opool,
            tc.tile_pool(name="ps", bufs=8, space="PSUM") as pspool,
        ):
            s_sb = cpool.tile([128, 500], BF16)
            nc.sync.dma_start(s_sb[:], s[:])
            s_ea = s_sb[:, 0:125]
            s_ed = s_sb[:, 125:250]
            s_oa = s_sb[:, 250:375]
            s_od = s_sb[:, 375:500]

            for i in range(windows):
                xt = xpool.tile([128, NB, CIN], BF16)
                nc.sync.dma_start(
                    xt[:], x[:, 125 * i : 125 * i + 128, :].rearrange("b r c -> r b c")
                )
                a_h = [xt[:, 8 * h : 8 * h + 8, 0:C] for h in range(2)]
                d_h = [xt[:, 8 * h : 8 * h + 8, C:CIN] for h in range(2)]
                ps_e = [pspool.tile([128, 8, C], F32, tag="ps", name=f"ps_e{i}_{h}") for h in range(2)]
                ps_o = [pspool.tile([128, 8, C], F32, tag="ps", name=f"ps_o{i}_{h}") for h in range(2)]
                # weight-paired order: 4 stationary loads per window
                for h in range(2):
                    nc.tensor.matmul(ps_e[h][0:125], s_ea, a_h[h], start=True, stop=False)
                for h in range(2):
                    nc.tensor.matmul(ps_e[h][0:125], s_ed, d_h[h], start=False, stop=True)
                for h in range(2):
                    nc.tensor.matmul(ps_o[h][0:125], s_oa, a_h[h], start=True, stop=False)
                for h in range(2):
                    nc.tensor.matmul(ps_o[h][0:125], s_od, d_h[h], start=False, stop=True)

                ot = opool.tile([128, NB, 2, C], BF16)
                nc.scalar.copy(ot[0:125, 0:8, 0, :], ps_e[0][0:125])
                nc.scalar.copy(ot[0:125, 0:8, 1, :], ps_o[0][0:125])
                nc.vector.tensor_copy(ot[0:125, 8:16, 0, :], ps_e[1][0:125])
                nc.vector.tensor_copy(ot[0:125, 8:16, 1, :], ps_o[1][0:125])

                nc.scalar.dma_start(
                    y[:, 250 * i : 250 * i + 250, :].rearrange(
                        "b (q two) c -> q b (two c)", two=2
                    ),
                    ot[0:125].rearrange("p b s c -> p b (s c)"),
                )
    _install_wait_splitter(nc)
    return nc


def _install_wait_splitter(nc):
    """This walrus build encodes at most one sync wait per instruction. Split
    every multi-wait instruction in the serialized BIR into single-wait NoOps
    placed immediately before it on the same engine (in-order semantics are
    identical)."""
    import orjson

    orig = nc.to_json_bytes

    def patched():
        d = orjson.loads(orig())
        n_split = 0
        for fn in d["functions"]:
            for bb in fn["blocks"]:
                out = []
                for inst in bb["instructions"]:
                    si = inst.get("sync_info")
                    waits = si.get("on_wait", []) if si else []
                    if len(waits) > 1:
                        for j, w in enumerate(waits[:-1]):
                            out.append(
                                {
                                    "debug": inst.get("debug", 0),
                                    "engine": inst["engine"],
                                    "ins": [],
                                    "name": f"{inst['name']}_sw{j}",
                                    "opcode": "NoOp",
                                    "outs": [],
                                    "sync_info": {
                                        "on_update": [],
                                        "on_wait": [w],
                                    },
                                    "text_hint": "split_wait",
                                }
                            )
                            n_split += 1
                        si["on_wait"] = [waits[-1]]
                    out.append(inst)
                bb["instructions"] = out
        return orjson.dumps(d)

    nc.to_json_bytes = patched


# ---------------------------------------------------------------------------
# Fast PJRT execution path: functionally identical to
# concourse.bass2jax.run_bass_via_pjrt, but (a) the traced/lowered/compiled
# executable is memoized per Bass program instead of being rebuilt (and
# recompiled) on every call, (b) the donated output buffers are created
# on-device by a tiny jitted fill instead of uploading host zero arrays
# through the tunnel, (c) inputs are uploaded with an explicit sharded
# device_put (faster than the jit argument path; small constant inputs stay
# device-resident keyed by content), and (d) outputs are fetched shard-wise
# on a thread pool, optionally cast-assigned straight into a caller-provided
# sink buffer. Data semantics per call are unchanged: inputs are uploaded,
# the NEFF runs on all cores, outputs are fetched.
# ---------------------------------------------------------------------------

_PJRT_CACHE = {}
_PJRT_LOCK = threading.Lock()
_TLS = threading.local()


def _fast_run_bass_via_pjrt(nc, in_maps, n_cores):
    import jax
    import jax.numpy as jnp
    from jax.sharding import Mesh, NamedSharding, PartitionSpec
    from jax.experimental.shard_map import shard_map
    from concourse import bass2jax

    if nc.dbg_addr is not None or n_cores == 1:
        return _ORIG_RUN_VIA_PJRT(nc, in_maps, n_cores)

    with _PJRT_LOCK:
        entry = _PJRT_CACHE.get(id(nc))
        if entry is None:
            bass2jax.install_neuronx_cc_hook()
            partition_name = (
                nc.partition_id_tensor.name if nc.partition_id_tensor else None
            )
            in_names, out_names, out_avals = [], [], []
            for alloc in nc.m.functions[0].allocations:
                if not isinstance(alloc, mybir.MemoryLocationSet):
                    continue
                assert alloc.memorylocations
                name = alloc.memorylocations[0].name
                if alloc.kind == "ExternalInput":
                    if name != partition_name:
                        in_names.append(name)
                elif alloc.kind == "ExternalOutput":
                    assert alloc.tensor_shape is not None and alloc.dtype is not None
                    out_names.append(name)
                    out_avals.append(
                        jax.core.ShapedArray(
                            tuple(alloc.tensor_shape), mybir.dt.np(alloc.dtype)
                        )
                    )
            n_params = len(in_names)
            n_outs = len(out_avals)
            all_in_names = in_names + out_names
            if partition_name is not None:
                all_in_names.append(partition_name)
            donate = tuple(range(n_params, n_params + n_outs))

            def _body(*args):
                operands = list(args)
                if partition_name is not None:
                    operands.append(bass2jax.partition_id_tensor())
                outs = bass2jax._bass_exec_p.bind(
                    *operands,
                    out_avals=tuple(out_avals),
                    in_names=tuple(all_in_names),
                    out_names=tuple(out_names),
                    lowering_input_output_aliases=(),
                    sim_require_finite=True,
                    sim_require_nnan=True,
                    nc=nc,
                )
                return tuple(outs)

            devices = jax.devices()[:n_cores]
            assert len(devices) == n_cores
            mesh = Mesh(np.asarray(devices), ("core",))
            in_specs = (PartitionSpec("core"),) * (n_params + n_outs)
            out_specs = (PartitionSpec("core"),) * n_outs
            sharded = jax.jit(
                shard_map(
                    _body,
                    mesh=mesh,
                    in_specs=in_specs,
                    out_specs=out_specs,
                    check_rep=False,
                ),
                donate_argnums=donate,
                keep_unused=True,
            )
            gshapes = [(n_cores * av.shape[0], *av.shape[1:]) for av in out_avals]
            gdtypes = [av.dtype for av in out_avals]
            shd = NamedSharding(mesh, PartitionSpec("core"))
            make_zeros = jax.jit(
                lambda: tuple(jnp.zeros(gs, gd) for gs, gd in zip(gshapes, gdtypes)),
                out_shardings=(shd,) * n_outs,
            )
            entry = (in_names, out_names, out_avals, sharded, make_zeros, shd, {})
            _PJRT_CACHE[id(nc)] = entry

    in_names, out_names, out_avals, sharded, make_zeros, shd, small_cache = entry
    import os as _os
    import time as _time

    _timing = _os.environ.get("KM_TIMING", "0") == "1"
    _t0 = _time.time()
    concat_in = [
        _concat_or_base([np.asarray(m[name]) for m in in_maps]) for name in in_names
    ]
    dev_in = []
    for name, arr in zip(in_names, concat_in):
        if arr.nbytes <= (4 << 20):
            key = (name, arr.shape, arr.dtype.str, arr.tobytes())
            buf = small_cache.get(key)
            if buf is None:
                small_cache.clear()
                buf = jax.device_put(arr, shd)
                small_cache[key] = buf
            dev_in.append(buf)
        else:
            dev_in.append(jax.device_put(arr, shd))
    zeros = make_zeros()
    _t1 = _time.time()
    out_arrs = sharded(*dev_in, *zeros)
    cb = getattr(_TLS, "post_dispatch", None)
    if cb is not None:
        cb()
    for o in out_arrs:
        o.block_until_ready()
    _t2 = _time.time()
    fetched = [
        _fetch_sharded(o, av, n_cores, nm)
        for o, av, nm in zip(out_arrs, out_avals, out_names)
    ]
    _t3 = _time.time()
    if _timing:
        print(
            f"[km] upload+dispatch={_t1 - _t0:.3f}s exec={_t2 - _t1:.3f}s "
            f"fetch={_t3 - _t2:.3f}s"
        )
    return [
        {name: fetched[i][c] for i, name in enumerate(out_names)}
        for c in range(n_cores)
    ]


def _concat_or_base(arrs):
    """Concatenate per-core arrays along axis 0 — zero-copy when they are
    consecutive contiguous views of one base array of exactly that shape."""
    base = arrs[0].base
    if (
        base is not None
        and all(a.base is base for a in arrs)
        and all(a.flags["C_CONTIGUOUS"] for a in arrs)
        and base.flags["C_CONTIGUOUS"]
        and base.dtype == arrs[0].dtype
        and base.size == sum(a.size for a in arrs)
    ):
        ptr = base.__array_interface__["data"][0]
        ok = True
        for a in arrs:
            if a.__array_interface__["data"][0] != ptr:
                ok = False
                break
            ptr += a.nbytes
        if ok:
            gshape = (sum(a.shape[0] for a in arrs),) + arrs[0].shape[1:]
            return base.reshape(gshape)
    return np.concatenate(arrs, axis=0)


_FETCH_POOL = None
_FETCH_POOL_LOCK = threading.Lock()


def _get_fetch_pool():
    global _FETCH_POOL
    from concurrent.futures import ThreadPoolExecutor

    with _FETCH_POOL_LOCK:
        if _FETCH_POOL is None:
            _FETCH_POOL = ThreadPoolExecutor(2 * N_CORES)
    return _FETCH_POOL


def _fetch_sharded(arr, aval, n_cores, name):
    """Fetch a fully core-sharded device array with one task per shard
    (overlaps the per-shard tunnel round-trips). When the calling thread has
    a fetch sink installed, each shard is cast-assigned straight into the
    caller's buffer inside the fetch task. Returns a per-core indexable."""
    sink_map = getattr(_TLS, "sink", None)
    sink = sink_map.get(name) if sink_map else None
    shards = sorted(arr.addressable_shards, key=lambda s: s.index[0].start or 0)
    if len(shards) != n_cores:
        return np.asarray(arr).reshape(n_cores, *aval.shape)
    pool = _get_fetch_pool()
    if sink is None:
        out = np.empty((n_cores, *aval.shape), aval.dtype)

        def _pull(c):
            out[c] = np.asarray(shards[c].data)

        list(pool.map(_pull, range(n_cores)))
        return out

    res = [None] * n_cores

    def _pull_sink(c):
        dst = sink(c)
        dst[...] = np.asarray(shards[c].data)
        res[c] = dst

    list(pool.map(_pull_sink, range(n_cores)))
    return res


_ORIG_RUN_VIA_PJRT = None


def _install_fast_pjrt():
    global _ORIG_RUN_VIA_PJRT
    from concourse import bass2jax

    if _ORIG_RUN_VIA_PJRT is None:
        _ORIG_RUN_VIA_PJRT = bass2jax.run_bass_via_pjrt
        bass2jax.run_bass_via_pjrt = _fast_run_bass_via_pjrt


_PROGRAMS = {}


def _get_nc(rows_per_core):
    nc = _PROGRAMS.get(rows_per_core)
    if nc is None:
        nc = _build_program(rows_per_core)
        _PROGRAMS[rows_per_core] = nc
    return nc


def _band_matrices(rec_lo: np.ndarray, rec_hi: np.ndarray) -> np.ndarray:
    """[128, 500] = [S_even_approx | S_even_detail | S_odd_approx | S_odd_detail].

    S[k, m]: coefficient linking input row r0+k to output pair m of a window
    (k = m + j, j = 0..3). Even phase uses taps f[2j+1], odd phase f[2j]."""
    s = np.zeros((128, 500), np.float32)
    lo = np.asarray(rec_lo, np.float32)
    hi = np.asarray(rec_hi, np.float32)
    for m in range(125):
        for j in range(4):
            k = m + j
            s[k, m] = lo[2 * j + 1]
            s[k, 125 + m] = hi[2 * j + 1]
            s[k, 250 + m] = lo[2 * j]
            s[k, 375 + m] = hi[2 * j]
    return s.astype(NP_BF16)


def _shard_inputs_chunk(x_f32, s, chunk, chunks):
    """Per-core input maps for one L chunk: rows_per_core+3 rows each (1-row
    left halo, 2-row right), zero-padded at the global edges, cast f32->bf16
    during the slab build. The per-core slabs are consecutive views of one
    preallocated global array so the pjrt path can skip its concat."""
    rows = L // (N_CORES * chunks)
    xrows = rows + 3
    xg = np.empty((N_CORES * NB, xrows, CIN), NP_BF16)
    for core in range(N_CORES):
        r0 = (L // chunks) * chunk + rows * core
        xc = xg[core * NB : (core + 1) * NB]
        lo = r0 - 1
        hi = r0 + rows + 2  # exclusive; 2-row right halo
        if lo < 0:
            xc[:, 0, :] = 0
            xc[:, 1:, :] = x_f32[:, 0:hi, :]
        elif hi > L:
            n = L - lo
            xc[:, :n, :] = x_f32[:, lo:L, :]
            xc[:, n:, :] = 0
        else:
            xc[:] = x_f32[:, lo:hi, :]
    return [
        {"x": xg[core * NB : (core + 1) * NB], "s": s} for core in range(N_CORES)
    ]


_CHUNK_POOL = None


def _get_chunk_pool():
    global _CHUNK_POOL
    from concurrent.futures import ThreadPoolExecutor

    if _CHUNK_POOL is None:
        _CHUNK_POOL = ThreadPoolExecutor(4)
    return _CHUNK_POOL


def kernel(inputs: np.ndarray, rec_lo: np.ndarray, rec_hi: np.ndarray) -> np.ndarray:
    import os as _os
    import time as _time

    _timing = _os.environ.get("KM_TIMING", "0") == "1"
    chunks = CHUNKS
    _t0 = _time.time()
    inputs = np.asarray(inputs, np.float32)
    assert inputs.shape == (NB, L, CIN), inputs.shape
    _install_fast_pjrt()
    rows = L // (N_CORES * chunks)
    nc = _get_nc(rows)
    s = _band_matrices(rec_lo, rec_hi)
    in_chunks = [_shard_inputs_chunk(inputs, s, k, chunks) for k in range(chunks)]
    out = np.empty((NB, 2 * L, C), np.float32)

    def _dst(chunk, core):
        start = 2 * ((L // chunks) * chunk + rows * core)
        return out[:, start : start + 2 * rows, :]

    _t1 = _time.time()
    results = [None] * chunks
    if chunks == 1:
        _TLS.sink = {"y": lambda c: _dst(0, c)}
        _TLS.post_dispatch = None
        try:
            results[0] = run_bass_kernel_spmd(nc, in_chunks[0], list(range(N_CORES)))
        finally:
            _TLS.sink = None
    else:
        # Pipeline the chunks on threads: chunk k+1 may start its upload as
        # soon as chunk k's exec has been dispatched, so its transfer
        # overlaps chunk k's fetch.
        events = [threading.Event() for _ in range(chunks)]

        def _run(k):
            if k > 0:
                events[k - 1].wait()
            _TLS.sink = {"y": lambda c, k=k: _dst(k, c)}
            _TLS.post_dispatch = events[k].set
            try:
                return run_bass_kernel_spmd(nc, in_chunks[k], list(range(N_CORES)))
            finally:
                events[k].set()
                _TLS.sink = None
                _TLS.post_dispatch = None

        pool = _get_chunk_pool()
        futs = [pool.submit(_run, k) for k in range(chunks)]
        results = [f.result() for f in futs]
    _t2 = _time.time()
    # If a fetch sink was honored, the per-core results are views into `out`
    # already; otherwise (fallback paths) cast-assign them now.
    for k in range(chunks):
        for core in range(N_CORES):
            got = results[k].results[core]["y"]
            if not np.shares_memory(got, out):
                _dst(k, core)[...] = got
    _t3 = _time.time()
    if _timing:
        print(
            f"[km kernel] prep={_t1 - _t0:.3f}s spmd={_t2 - _t1:.3f}s "
            f"reassemble={_t3 - _t2:.3f}s"
        )
    return out


# revision 16
# speedup vs baseline: 1.5758x; 1.2508x over previous
"""Inverse Daubechies (db4) wavelet layer on 8 Trainium2 NeuronCores.

Math: input [16, 16000, 128] splits into approx (ch 0:64) / detail (ch 64:128).
Each half is zero-upsampled 2x along L and cross-correlated with an 8-tap
filter (TF SAME padding, pad_left=3), outputs summed -> [16, 32000, 64].

Polyphase view: out[2t]   = sum_j rec[2j+1] * z[t+j-1]
               out[2t+1] = sum_j rec[2j]   * z[t+j-1]        (j = 0..3)
summed over both halves (rec_lo on approx + rec_hi on detail).

Kernel strategy (per core, per chunk): shard L across the 8 cores; each
chunk covers ROWS_PER_CORE input rows per core. The upsample+conv+sum is
expressed as PE matmuls with banded stationary matrices: partition dim =
input L-rows (K=128 window), free dim = (batch, channel) (N=512), M = 125
output positions per phase. PSUM accumulation fuses the approx+detail sum.
Even/odd phases are copied into an SBUF tile so each partition holds a
consecutive output row pair -> fully contiguous per-partition DMA to DRAM.

Host/transfer strategy: the end-to-end latency is dominated by the
host<->device tunnel, so I/O is bf16 end to end (inputs quantized during
the shard build, outputs cast to f32 inside the fetch threads), the PJRT
execution path is memoized per program, donated output buffers are created
device-side, and the L range is split into CHUNKS sequential spmd calls
pipelined on two threads so chunk k+1's upload overlaps chunk k's fetch.
"""

import threading
import numpy as np
import ml_dtypes

import concourse.bass as bass
import concourse.tile as tile
from concourse import mybir
from concourse.bass_utils import run_bass_kernel_spmd
from concourse.vector_clock import ScopedClock, VectorClock

F32 = mybir.dt.float32
BF16 = mybir.dt.bfloat16
NP_BF16 = ml_dtypes.bfloat16

N_CORES = 8
NB = 16        # batches
CIN = 128      # input channels (64 approx + 64 detail)
C = 64         # output channels
L = 16000      # input length
CHUNKS = 2     # sequential pipelined spmd calls over the L axis


class _TileContextFixed(tile.TileContext):
    """This walrus build only encodes one sync wait per instruction; Tile's
    final drain carries one wait per logical proc. Split them into
    single-wait nops ahead of a waitless drain."""

    def _drain_and_barrier(self, tick_clock, wait_clock):
        nc = self.nc
        gc = tick_clock.global_clock
        n = len(gc)
        for p in range(n):
            t = gc[p]
            if t <= 0:
                continue
            vec = [0] * n
            vec[p] = t
            nop = nc.sync.nop(nofuse=True, hint=f"drain_wait_p{p}")
            wait_clock.add_sem_waits(nop.ins, ScopedClock({None: VectorClock(vec)}))
        nc.sync.drain()
        nc.all_engine_barrier()
        assert self.sems is not None
        popped = nc._tile_sem_poison_stack.pop()
        assert popped is self._sem_poison
        nc.clear_and_free_semaphores(list(self.sems.allocated().values()))
        nc.all_engine_barrier()


def _build_program(rows_per_core):
    windows = rows_per_core // 125
    xrows = rows_per_core + 3
    nc = bass.Bass(
        trn_type="TRN2", target_bir_lowering=False, debug=False, num_devices=N_CORES
    )
    x = nc.dram_tensor("x", (NB, xrows, CIN), BF16, kind="ExternalInput")
    s = nc.dram_tensor("s", (128, 500), BF16, kind="ExternalInput")
    y = nc.dram_tensor("y", (NB, 2 * rows_per_core, C), BF16, kind="ExternalOutput")

    with _TileContextFixed(nc) as tc:
        with (
            tc.tile_pool(name="const", bufs=1) as cpool,
            tc.tile_pool(name="xin", bufs=3) as xpool,
            tc.tile_pool(name="outb", bufs=3) as opool,
            tc.tile_pool(name="ps", bufs=8, space="PSUM") as pspool,
        ):
            s_sb = cpool.tile([128, 500], BF16)
            nc.sync.dma_start(s_sb[:], s[:])
            s_ea = s_sb[:, 0:125]
            s_ed = s_sb[:, 125:250]
            s_oa = s_sb[:, 250:375]
            s_od = s_sb[:, 375:500]

            for i in range(windows):
                xt = xpool.tile([128, NB, CIN], BF16)
                nc.sync.dma_start(
                    xt[:], x[:, 125 * i : 125 * i + 128, :].rearrange("b r c -> r b c")
                )
                a_h = [xt[:, 8 * h : 8 * h + 8, 0:C] for h in range(2)]
                d_h = [xt[:, 8 * h : 8 * h + 8, C:CIN] for h in range(2)]
                ps_e = [pspool.tile([128, 8, C], F32, tag="ps", name=f"ps_e{i}_{h}") for h in range(2)]
                ps_o = [pspool.tile([128, 8, C], F32, tag="ps", name=f"ps_o{i}_{h}") for h in range(2)]
                # weight-paired order: 4 stationary loads per window
                for h in range(2):
                    nc.tensor.matmul(ps_e[h][0:125], s_ea, a_h[h], start=True, stop=False)
                for h in range(2):
                    nc.tensor.matmul(ps_e[h][0:125], s_ed, d_h[h], start=False, stop=True)
                for h in range(2):
                    nc.tensor.matmul(ps_o[h][0:125], s_oa, a_h[h], start=True, stop=False)
                for h in range(2):
                    nc.tensor.matmul(ps_o[h][0:125], s_od, d_h[h], start=False, stop=True)

                ot = opool.tile([128, NB, 2, C], BF16)
                nc.scalar.copy(ot[0:125, 0:8, 0, :], ps_e[0][0:125])
                nc.scalar.copy(ot[0:125, 0:8, 1, :], ps_o[0][0:125])
                nc.vector.tensor_copy(ot[0:125, 8:16, 0, :], ps_e[1][0:125])
                nc.vector.tensor_copy(ot[0:125, 8:16, 1, :], ps_o[1][0:125])

                nc.scalar.dma_start(
                    y[:, 250 * i : 250 * i + 250, :].rearrange(
                        "b (q two) c -> q b (two c)", two=2
                    ),
                    ot[0:125].rearrange("p b s c -> p b (s c)"),
                )
    _install_wait_splitter(nc)
    return nc


def _install_wait_splitter(nc):
    """This walrus build encodes at most one sync wait per instruction. Split
    every multi-wait instruction in the serialized BIR into single-wait NoOps
    placed immediately before it on the same engine (in-order semantics are
    identical)."""
    import orjson

    orig = nc.to_json_bytes

    def patched():
        d = orjson.loads(orig())
        n_split = 0
        for fn in d["functions"]:
            for bb in fn["blocks"]:
                out = []
                for inst in bb["instructions"]:
                    si = inst.get("sync_info")
                    waits = si.get("on_wait", []) if si else []
                    if len(waits) > 1:
                        for j, w in enumerate(waits[:-1]):
                            out.append(
                                {
                                    "debug": inst.get("debug", 0),
                                    "engine": inst["engine"],
                                    "ins": [],
                                    "name": f"{inst['name']}_sw{j}",
                                    "opcode": "NoOp",
                                    "outs": [],
                                    "sync_info": {
                                        "on_update": [],
                                        "on_wait": [w],
                                    },
                                    "text_hint": "split_wait",
                                }
                            )
                            n_split += 1
                        si["on_wait"] = [waits[-1]]
                    out.append(inst)
                bb["instructions"] = out
        return orjson.dumps(d)

    nc.to_json_bytes = patched


# ---------------------------------------------------------------------------
# Fast PJRT execution path: functionally identical to
# concourse.bass2jax.run_bass_via_pjrt, but (a) the traced/lowered/compiled
# executable is memoized per Bass program instead of being rebuilt (and
# recompiled) on every call, (b) the donated output buffers are created
# on-device by a tiny jitted fill instead of uploading host zero arrays
# through the tunnel, (c) inputs are uploaded with an explicit sharded
# device_put (faster than the jit argument path; small constant inputs stay
# device-resident keyed by content), and (d) outputs are fetched shard-wise
# on a thread pool, optionally cast-assigned straight into a caller-provided
# sink buffer. Data semantics per call are unchanged: inputs are uploaded,
# the NEFF runs on all cores, outputs are fetched.
# ---------------------------------------------------------------------------

_PJRT_CACHE = {}
_PJRT_LOCK = threading.Lock()
_TLS = threading.local()


def _fast_run_bass_via_pjrt(nc, in_maps, n_cores):
    import jax
    import jax.numpy as jnp
    from jax.sharding import Mesh, NamedSharding, PartitionSpec
    from jax.experimental.shard_map import shard_map
    from concourse import bass2jax

    if nc.dbg_addr is not None or n_cores == 1:
        return _ORIG_RUN_VIA_PJRT(nc, in_maps, n_cores)

    with _PJRT_LOCK:
        entry = _PJRT_CACHE.get(id(nc))
        if entry is None:
            bass2jax.install_neuronx_cc_hook()
            partition_name = (
                nc.partition_id_tensor.name if nc.partition_id_tensor else None
            )
            in_names, out_names, out_avals = [], [], []
            for alloc in nc.m.functions[0].allocations:
                if not isinstance(alloc, mybir.MemoryLocationSet):
                    continue
                assert alloc.memorylocations
                name = alloc.memorylocations[0].name
                if alloc.kind == "ExternalInput":
                    if name != partition_name:
                        in_names.append(name)
                elif alloc.kind == "ExternalOutput":
                    assert alloc.tensor_shape is not None and alloc.dtype is not None
                    out_names.append(name)
                    out_avals.append(
                        jax.core.ShapedArray(
                            tuple(alloc.tensor_shape), mybir.dt.np(alloc.dtype)
                        )
                    )
            n_params = len(in_names)
            n_outs = len(out_avals)
            all_in_names = in_names + out_names
            if partition_name is not None:
                all_in_names.append(partition_name)
            donate = tuple(range(n_params, n_params + n_outs))

            def _body(*args):
                operands = list(args)
                if partition_name is not None:
                    operands.append(bass2jax.partition_id_tensor())
                outs = bass2jax._bass_exec_p.bind(
                    *operands,
                    out_avals=tuple(out_avals),
                    in_names=tuple(all_in_names),
                    out_names=tuple(out_names),
                    lowering_input_output_aliases=(),
                    sim_require_finite=True,
                    sim_require_nnan=True,
                    nc=nc,
                )
                return tuple(outs)

            devices = jax.devices()[:n_cores]
            assert len(devices) == n_cores
            mesh = Mesh(np.asarray(devices), ("core",))
            in_specs = (PartitionSpec("core"),) * (n_params + n_outs)
            out_specs = (PartitionSpec("core"),) * n_outs
            sharded = jax.jit(
                shard_map(
                    _body,
                    mesh=mesh,
                    in_specs=in_specs,
                    out_specs=out_specs,
                    check_rep=False,
                ),
                donate_argnums=donate,
                keep_unused=True,
            )
            gshapes = [(n_cores * av.shape[0], *av.shape[1:]) for av in out_avals]
            gdtypes = [av.dtype for av in out_avals]
            shd = NamedSharding(mesh, PartitionSpec("core"))
            make_zeros = jax.jit(
                lambda: tuple(jnp.zeros(gs, gd) for gs, gd in zip(gshapes, gdtypes)),
                out_shardings=(shd,) * n_outs,
            )
            entry = (in_names, out_names, out_avals, sharded, make_zeros, shd, {})
            _PJRT_CACHE[id(nc)] = entry

    in_names, out_names, out_avals, sharded, make_zeros, shd, small_cache = entry
    import os as _os
    import time as _time

    _timing = _os.environ.get("KM_TIMING", "0") == "1"
    _t0 = _time.time()
    concat_in = [
        _concat_or_base([np.asarray(m[name]) for m in in_maps]) for name in in_names
    ]
    dev_in = []
    for name, arr in zip(in_names, concat_in):
        if arr.nbytes <= (4 << 20):
            key = (name, arr.shape, arr.dtype.str, arr.tobytes())
            buf = small_cache.get(key)
            if buf is None:
                small_cache.clear()
                buf = jax.device_put(arr, shd)
                small_cache[key] = buf
            dev_in.append(buf)
        else:
            dev_in.append(jax.device_put(arr, shd))
    zeros = make_zeros()
    _t1 = _time.time()
    out_arrs = sharded(*dev_in, *zeros)
    cb = getattr(_TLS, "post_dispatch", None)
    if cb is not None:
        cb()
    # No barrier here: each fetch task blocks on its own shard, so shards of
    # early-finishing devices download while later devices still execute.
    _t2 = _time.time()
    fetched = [
        _fetch_sharded(o, av, n_cores, nm)
        for o, av, nm in zip(out_arrs, out_avals, out_names)
    ]
    _t3 = _time.time()
    if _timing:
        print(
            f"[km] upload+dispatch={_t1 - _t0:.3f}s exec={_t2 - _t1:.3f}s "
            f"fetch={_t3 - _t2:.3f}s"
        )
    return [
        {name: fetched[i][c] for i, name in enumerate(out_names)}
        for c in range(n_cores)
    ]


def _concat_or_base(arrs):
    """Concatenate per-core arrays along axis 0 — zero-copy when they are
    consecutive contiguous views of one base array of exactly that shape."""
    base = arrs[0].base
    if (
        base is not None
        and all(a.base is base for a in arrs)
        and all(a.flags["C_CONTIGUOUS"] for a in arrs)
        and base.flags["C_CONTIGUOUS"]
        and base.dtype == arrs[0].dtype
        and base.size == sum(a.size for a in arrs)
    ):
        ptr = base.__array_interface__["data"][0]
        ok = True
        for a in arrs:
            if a.__array_interface__["data"][0] != ptr:
                ok = False
                break
            ptr += a.nbytes
        if ok:
            gshape = (sum(a.shape[0] for a in arrs),) + arrs[0].shape[1:]
            return base.reshape(gshape)
    return np.concatenate(arrs, axis=0)


_FETCH_POOL = None
_FETCH_POOL_LOCK = threading.Lock()


def _get_fetch_pool():
    global _FETCH_POOL
    from concurrent.futures import ThreadPoolExecutor

    with _FETCH_POOL_LOCK:
        if _FETCH_POOL is None:
            _FETCH_POOL = ThreadPoolExecutor(2 * N_CORES)
    return _FETCH_POOL


def _fetch_sharded(arr, aval, n_cores, name):
    """Fetch a fully core-sharded device array with one task per shard
    (overlaps the per-shard tunnel round-trips). When the calling thread has
    a fetch sink installed, each shard is cast-assigned straight into the
    caller's buffer inside the fetch task. Returns a per-core indexable."""
    sink_map = getattr(_TLS, "sink", None)
    sink = sink_map.get(name) if sink_map else None
    shards = sorted(arr.addressable_shards, key=lambda s: s.index[0].start or 0)
    if len(shards) != n_cores:
        return np.asarray(arr).reshape(n_cores, *aval.shape)
    pool = _get_fetch_pool()
    if sink is None:
        out = np.empty((n_cores, *aval.shape), aval.dtype)

        def _pull(c):
            out[c] = np.asarray(shards[c].data)

        list(pool.map(_pull, range(n_cores)))
        return out

    res = [None] * n_cores

    def _pull_sink(c):
        dst = sink(c)
        dst[...] = np.asarray(shards[c].data)
        res[c] = dst

    list(pool.map(_pull_sink, range(n_cores)))
    return res


_ORIG_RUN_VIA_PJRT = None


def _install_fast_pjrt():
    global _ORIG_RUN_VIA_PJRT
    from concourse import bass2jax

    if _ORIG_RUN_VIA_PJRT is None:
        _ORIG_RUN_VIA_PJRT = bass2jax.run_bass_via_pjrt
        bass2jax.run_bass_via_pjrt = _fast_run_bass_via_pjrt


_PROGRAMS = {}


def _get_nc(rows_per_core):
    nc = _PROGRAMS.get(rows_per_core)
    if nc is None:
        nc = _build_program(rows_per_core)
        _PROGRAMS[rows_per_core] = nc
    return nc


def _band_matrices(rec_lo: np.ndarray, rec_hi: np.ndarray) -> np.ndarray:
    """[128, 500] = [S_even_approx | S_even_detail | S_odd_approx | S_odd_detail].

    S[k, m]: coefficient linking input row r0+k to output pair m of a window
    (k = m + j, j = 0..3). Even phase uses taps f[2j+1], odd phase f[2j]."""
    s = np.zeros((128, 500), np.float32)
    lo = np.asarray(rec_lo, np.float32)
    hi = np.asarray(rec_hi, np.float32)
    for m in range(125):
        for j in range(4):
            k = m + j
            s[k, m] = lo[2 * j + 1]
            s[k, 125 + m] = hi[2 * j + 1]
            s[k, 250 + m] = lo[2 * j]
            s[k, 375 + m] = hi[2 * j]
    return s.astype(NP_BF16)


def _shard_inputs_chunk(x_f32, s, chunk, chunks):
    """Per-core input maps for one L chunk: rows_per_core+3 rows each (1-row
    left halo, 2-row right), zero-padded at the global edges, cast f32->bf16
    during the slab build. The per-core slabs are consecutive views of one
    preallocated global array so the pjrt path can skip its concat."""
    rows = L // (N_CORES * chunks)
    xrows = rows + 3
    xg = np.empty((N_CORES * NB, xrows, CIN), NP_BF16)
    for core in range(N_CORES):
        r0 = (L // chunks) * chunk + rows * core
        xc = xg[core * NB : (core + 1) * NB]
        lo = r0 - 1
        hi = r0 + rows + 2  # exclusive; 2-row right halo
        if lo < 0:
            xc[:, 0, :] = 0
            xc[:, 1:, :] = x_f32[:, 0:hi, :]
        elif hi > L:
            n = L - lo
            xc[:, :n, :] = x_f32[:, lo:L, :]
            xc[:, n:, :] = 0
        else:
            xc[:] = x_f32[:, lo:hi, :]
    return [
        {"x": xg[core * NB : (core + 1) * NB], "s": s} for core in range(N_CORES)
    ]


_CHUNK_POOL = None


def _get_chunk_pool():
    global _CHUNK_POOL
    from concurrent.futures import ThreadPoolExecutor

    if _CHUNK_POOL is None:
        _CHUNK_POOL = ThreadPoolExecutor(4)
    return _CHUNK_POOL


def kernel(inputs: np.ndarray, rec_lo: np.ndarray, rec_hi: np.ndarray) -> np.ndarray:
    import os as _os
    import time as _time

    _timing = _os.environ.get("KM_TIMING", "0") == "1"
    chunks = CHUNKS
    _t0 = _time.time()
    inputs = np.asarray(inputs, np.float32)
    assert inputs.shape == (NB, L, CIN), inputs.shape
    _install_fast_pjrt()
    rows = L // (N_CORES * chunks)
    nc = _get_nc(rows)
    s = _band_matrices(rec_lo, rec_hi)
    in_chunks = [_shard_inputs_chunk(inputs, s, k, chunks) for k in range(chunks)]
    out = np.empty((NB, 2 * L, C), np.float32)

    def _dst(chunk, core):
        start = 2 * ((L // chunks) * chunk + rows * core)
        return out[:, start : start + 2 * rows, :]

    _t1 = _time.time()
    results = [None] * chunks
    if chunks == 1:
        _TLS.sink = {"y": lambda c: _dst(0, c)}
        _TLS.post_dispatch = None
        try:
            results[0] = run_bass_kernel_spmd(nc, in_chunks[0], list(range(N_CORES)))
        finally:
            _TLS.sink = None
    else:
        # Pipeline the chunks on threads: chunk k+1 may start its upload as
        # soon as chunk k's exec has been dispatched, so its transfer
        # overlaps chunk k's fetch.
        events = [threading.Event() for _ in range(chunks)]

        def _run(k):
            if k > 0:
                events[k - 1].wait()
            _TLS.sink = {"y": lambda c, k=k: _dst(k, c)}
            _TLS.post_dispatch = events[k].set
            try:
                return run_bass_kernel_spmd(nc, in_chunks[k], list(range(N_CORES)))
            finally:
                events[k].set()
                _TLS.sink = None
                _TLS.post_dispatch = None

        pool = _get_chunk_pool()
        futs = [pool.submit(_run, k) for k in range(chunks)]
        results = [f.result() for f in futs]
    _t2 = _time.time()
    # If a fetch sink was honored, the per-core results are views into `out`
    # already; otherwise (fallback paths) cast-assign them now.
    for k in range(chunks):
        for core in range(N_CORES):
            got = results[k].results[core]["y"]
            if not np.shares_memory(got, out):
                _dst(k, core)[...] = got
    _t3 = _time.time()
    if _timing:
        print(
            f"[km kernel] prep={_t1 - _t0:.3f}s spmd={_t2 - _t1:.3f}s "
            f"reassemble={_t3 - _t2:.3f}s"
        )
    return out
